# revision 20
# baseline (speedup 1.0000x reference)
"""Raw-bass (manual semaphore) equivariant-linear kernel, v3.

Math: per head h, out[b,:,h::8] = M_h^T @ x[b,:,h::8] with M_h the
512x512 3D-circulant generated from (basis@kernel)[:,h]; only 4 distinct
128x128 blocks (d = (kc-mc) mod 4). One head per NeuronCore.

Findings this design encodes (from perfetto traces):
  - Each DGE ring feeds ~1 packet / ~40ns (per-partition-row packets), so
    input streaming is ring-dispatch-bound (~105 GB/s per ring), not HBM
    bound.  Three rings (SP + ACT HWDGE, POOL SWDGE) beat two.
  - The critical path is w + x0: x0 split across sync/scalar, w whole on
    gpsimd, so all three land ~11.3us and real matmuls start then.
  - PE HAM clock-gate: ~3.4us of sustained PE activity un-gates 1.2 ->
    2.4 GHz.  Dummy matmuls on garbage SBUF start right at block entry
    (~7.1us) so the flip happens before real matmuls begin.  Dummy output
    goes to PS[7], which every later real group overwrites with
    start=True.
  - PSUM column-split copies on TWO engines concurrently hang HW; the
    final group's copy is split into two *sequential* DVE copies instead,
    each chased by its own out-DMA on a different engine.
  - Output rows are 4KB (o16[mc] = (128, 2048)); DMA cost is per packet.

Layouts (per core):
  x16 (4 tb, 128, 2048) fp16 : row p = [kc0|kc1|kc2|kc3] tokens of block tb
  w16 (128, 512) fp16        : row p = [d0|d1|d2|d3]
  o16 (4 mc, 128, 2048) fp16 : row m = [tb0|tb1|tb2|tb3] (4KB rows)
"""

import os
from contextlib import ExitStack

import numpy as np

NUM_HEADS = 8
BATCH = 32
SEQ = 512
CHAN = 512
CH = CHAN // NUM_HEADS
P = 128
NKC = 4
NMC = 4
TOK = BATCH * CH
NTB = 4
N_WARM = 11
WARM_N = 512
SPLIT = 320  # first DVE copy of the final group covers cols [0:SPLIT]

LAST_RESULT = None
_BASS_CACHE = None


def _build_bass():
    import concourse.bass as bass
    import concourse.mybir as mybir

    fp16 = mybir.dt.float16
    fp32 = mybir.dt.float32

    nc = bass.Bass()

    x_d = nc.dram_tensor("x16", [NTB, P, NKC * 512], fp16, kind="ExternalInput")
    w_d = nc.dram_tensor("w16", [P, 4 * P], fp16, kind="ExternalInput")
    o_d = nc.dram_tensor("o16", [NMC, P, NTB * 512], fp16, kind="ExternalOutput")

    ctx = ExitStack()
    with ctx:
        XT = [
            ctx.enter_context(nc.sbuf_tensor(f"x_{tb}", [P, NKC * 512], fp16))
            for tb in range(NTB)
        ]
        WT = ctx.enter_context(nc.sbuf_tensor("w_all", [P, 4 * P], fp16))
        warm_w = ctx.enter_context(nc.sbuf_tensor("warm_w", [P, WARM_N], fp16))
        OT = [
            ctx.enter_context(nc.sbuf_tensor(f"ot_{mc}", [P, NTB * 512], fp16))
            for mc in range(NMC)
        ]
        PS = [
            ctx.enter_context(nc.psum_tensor(f"ps_{i}", [P, 512], fp32))
            for i in range(8)
        ]

        sem_mm = ctx.enter_context(nc.semaphore("mm"))
        sem_cp = ctx.enter_context(nc.semaphore("cp"))
        sem_cpa = ctx.enter_context(nc.semaphore("cpa"))
        sem_wa = ctx.enter_context(nc.semaphore("in_wa"))
        sem_wb = ctx.enter_context(nc.semaphore("in_wb"))
        sem_x0p = ctx.enter_context(nc.semaphore("in_x0p"))
        sem_x0g = ctx.enter_context(nc.semaphore("in_x0g"))
        sem_odg = ctx.enter_context(nc.semaphore("odg"))
        sem_x0c = ctx.enter_context(nc.semaphore("in_x0c"))
        sem_x1a = ctx.enter_context(nc.semaphore("in_x1a"))
        sem_x1b = ctx.enter_context(nc.semaphore("in_x1b"))
        sem_x2a = ctx.enter_context(nc.semaphore("in_x2a"))
        sem_x2b = ctx.enter_context(nc.semaphore("in_x2b"))
        sem_x3 = ctx.enter_context(nc.semaphore("in_x3"))
        sem_od = ctx.enter_context(nc.semaphore("od"))
        sem_warm = ctx.enter_context(nc.semaphore("warm"))

        # matmul schedule: (tb, d, mc, start, stop). d-major (weight reuse)
        # except the last tb, which is mc-major so the final psum groups
        # retire (and stream out) early.
        # entries: (tb, d, mc, start, stop, lo, hi) with [lo:hi) columns
        mm_order = []
        for tb in (0, 1, 2):
            for d in range(4):
                for mc in range(NMC):
                    mm_order.append((tb, d, mc, d == 0, d == 3, 0, 512))
        for mc in range(4):
            for d in range(4):
                mm_order.append((3, d, mc, d == 0, d == 3, 0, 512))

        def ps_tile(tb, mc):
            return PS[(tb % 2) * 4 + mc]

        grp_done = {}
        ngrp = 0
        for tb, d, mc, start, stop, lo, hi in mm_order:
            if stop:
                ngrp += 1
                grp_done[(tb, mc, lo)] = ngrp

        # psum->sbuf copies: tb 0..2 split DVE (mc even) / ACT (mc odd);
        # tb 3 all on DVE, with the final group done as two sequential
        # column-chunk copies (never two engines on one psum bank!).
        dve_copies = []  # (tb, mc, lo, hi)
        act_copies = []
        for tb in range(3):
            for mc in range(NMC):
                (dve_copies if mc % 2 == 0 else act_copies).append(
                    (tb, mc, 0, 512)
                )
        for mc in range(4):
            dve_copies.append((3, mc, 0, 512))

        cp_idx_d = {}
        for i, (tb, mc, lo, hi) in enumerate(dve_copies):
            cp_idx_d[(tb, mc, lo)] = i + 1
        cp_idx_a = {}
        for i, (tb, mc, lo, hi) in enumerate(act_copies):
            cp_idx_a[(tb, mc)] = i + 1

        with nc.Block() as block:

            @block.sync
            def _(sync):
                sync.dma_start(WT[:64], w_d[:64]).then_inc(sem_wa, 16)
                sync.dma_start(XT[0][:32], x_d[0][:32]).then_inc(sem_x0p, 16)
                sync.dma_start(XT[0][32:48], x_d[0][32:48]).then_inc(sem_x0p, 16)
                # keep the rings exclusive to x0 until it is nearly done
                sync.wait_ge(sem_x0p, 64)
                sync.dma_start(XT[1][:64], x_d[1][:64]).then_inc(sem_x1a, 16)
                sync.dma_start(XT[2][:64], x_d[2][:64]).then_inc(sem_x2a, 16)
                # out DMAs, mc 0,2 after their tb3 copies
                sync.wait_ge(sem_cp, cp_idx_d[(3, 0, 0)])
                sync.dma_start(o_d[0], OT[0][:]).then_inc(sem_od, 16)
                # final mc3: top partition half (parallel with scalar's
                # bottom half; partition splits stay on 64-boundaries)
                sync.wait_ge(sem_cpa, cp_idx_a[(2, 3)])
                sync.wait_ge(sem_cp, cp_idx_d[(3, 3, 0)])
                sync.dma_start(o_d[3, :64], OT[3][:64]).then_inc(sem_od, 16)

            @block.scalar
            def _(scalar):
                scalar.dma_start(WT[64:], w_d[64:]).then_inc(sem_wb, 16)
                scalar.dma_start(XT[0][48:64], x_d[0][48:64]).then_inc(sem_x0p, 16)
                scalar.dma_start(XT[0][64:80], x_d[0][64:80]).then_inc(sem_x0p, 16)
                scalar.wait_ge(sem_x0p, 64)
                scalar.dma_start(XT[1][64:], x_d[1][64:]).then_inc(sem_x1b, 16)
                scalar.dma_start(XT[2][64:], x_d[2][64:]).then_inc(sem_x2b, 16)
                # ACT copies for tb 0..2 (mc odd)
                for tb, mc, lo, hi in act_copies:
                    scalar.wait_ge(sem_mm, grp_done[(tb, mc, lo)])
                    nc.scalar.copy(
                        OT[mc][:, tb * 512 + lo : tb * 512 + hi],
                        ps_tile(tb, mc)[:, lo:hi],
                    ).then_inc(sem_cpa, 1)
                # out DMA mc 1 (tb0-2 cols of OT[1] are ACT copies)
                scalar.wait_ge(sem_cpa, cp_idx_a[(2, 1)])
                scalar.wait_ge(sem_cp, cp_idx_d[(3, 1, 0)])
                scalar.dma_start(o_d[1], OT[1][:]).then_inc(sem_od, 16)
                # final mc3: bottom partition half
                scalar.wait_ge(sem_cpa, cp_idx_a[(2, 3)])
                scalar.wait_ge(sem_cp, cp_idx_d[(3, 3, 0)])
                scalar.dma_start(o_d[3, 64:], OT[3][64:]).then_inc(sem_od, 16)
                # final mc3: bottom partition half
                scalar.wait_ge(sem_cpa, cp_idx_a[(2, 3)])
                scalar.wait_ge(sem_cp, cp_idx_d[(3, 3, 0)])
                scalar.dma_start(o_d[3, 64:], OT[3][64:]).then_inc(sem_od, 16)


            @block.gpsimd
            def _(gpsimd):
                gpsimd.dma_start(XT[0][80:112], x_d[0][80:112]).then_inc(
                    sem_x0g, 16
                )
                gpsimd.dma_start(XT[0][112:], x_d[0][112:]).then_inc(sem_x0g, 16)
                gpsimd.wait_ge(sem_x0p, 64)
                gpsimd.dma_start(XT[3][:], x_d[3]).then_inc(sem_x3, 16)
                # out DMA mc 2 rides gpsimd's otherwise-empty ring so the
                # final mc3 DMAs don't queue behind it
                gpsimd.wait_ge(sem_cp, cp_idx_d[(3, 2, 0)])
                gpsimd.dma_start(o_d[2], OT[2][:]).then_inc(sem_odg, 16)

            @block.tensor
            def _(tensor):
                # HAM warm-up: dummy matmuls from right after DVE's
                # memset (~7.3us) so the clock-gate sees PE activity while
                # inputs stream.  Results go to PS[7] and are never read;
                # every real group writing PS[7] opens with start=True.
                tensor.wait_ge(sem_warm, 1)
                for _ in range(N_WARM):
                    nc.tensor.matmul(
                        PS[7][:, :WARM_N],
                        warm_w[:, :P],
                        warm_w[:],
                        start=True,
                        stop=True,
                        skip_group_check=True,
                    )
                tensor.wait_ge(sem_wa, 16)
                tensor.wait_ge(sem_wb, 16)
                tensor.wait_ge(sem_x0p, 64)
                tensor.wait_ge(sem_x0g, 32)
                xsem = {1: (sem_x1a, sem_x1b), 2: (sem_x2a, sem_x2b),
                        3: (sem_x3,)}
                cur_tb = 0
                for tb, d, mc, start, stop, lo, hi in mm_order:
                    kc = (mc + d) % NKC
                    if tb != cur_tb:
                        for s in xsem[tb]:
                            tensor.wait_ge(s, 16)
                        if tb >= 2:
                            # WAR: psum banks reused from tb-2; wait for
                            # that tb's copies on both engines
                            tensor.wait_ge(
                                sem_cp,
                                max(
                                    cp_idx_d[(t, m, lo)]
                                    for (t, m, lo, hi) in dve_copies
                                    if t == tb - 2
                                ),
                            )
                            acts = [
                                cp_idx_a[(t, m)]
                                for (t, m, lo, hi) in act_copies
                                if t == tb - 2
                            ]
                            if acts:
                                tensor.wait_ge(sem_cpa, max(acts))
                        cur_tb = tb
                    mm = nc.tensor.matmul(
                        ps_tile(tb, mc)[:, lo:hi],
                        WT[:, d * P : (d + 1) * P],
                        XT[tb][:, kc * 512 + lo : kc * 512 + hi],
                        start=start,
                        stop=stop,
                        skip_group_check=True,
                    )
                    if stop:
                        mm.then_inc(sem_mm, 1)

            @block.vector
            def _(vector):
                nc.vector.memset(warm_w[:], 0.0).then_inc(sem_warm, 1)
                for tb, mc, lo, hi in dve_copies:
                    vector.wait_ge(sem_mm, grp_done[(tb, mc, lo)])
                    nc.vector.tensor_copy(
                        OT[mc][:, tb * 512 + lo : tb * 512 + hi],
                        ps_tile(tb, mc)[:, lo:hi],
                    ).then_inc(sem_cp, 1)

    return nc


def _weight_tiles(kexp_h):
    w3 = kexp_h.reshape(8, 8, 8)
    p = np.arange(P)
    m = np.arange(P)
    dj = ((p[:, None] // 8) % 8 - (m[None, :] // 8) % 8) % 8
    dk = (p[:, None] % 8 - m[None, :] % 8) % 8
    tiles = np.empty((4, P, P), np.float32)
    for d in range(4):
        di = (2 * d + p[:, None] // 64 - m[None, :] // 64) % 8
        tiles[d] = w3[di, dj, dk]
    return tiles


def _host_prep(x, kexp, h):
    xh = x[:, :, h::NUM_HEADS]  # (32, 512, 64)
    x_dev = (
        xh.transpose(1, 0, 2)        # (g'', b, c)
        .reshape(NKC, P, NTB, 512)   # (kc, p, tb, n)
        .transpose(2, 1, 0, 3)       # (tb, p, kc, n)
        .reshape(NTB, P, NKC * 512)
        .astype(np.float16)
    )
    w_dev = (
        _weight_tiles(kexp[:, h])    # (d, p, m)
        .transpose(1, 0, 2)          # (p, d, m)
        .reshape(P, 4 * P)
        .astype(np.float16)
    )
    return np.ascontiguousarray(x_dev), np.ascontiguousarray(w_dev)


def kernel(x, basis, kernel):
    global LAST_RESULT, _BASS_CACHE
    from concourse.bass_utils import run_bass_kernel_spmd

    x = np.ascontiguousarray(np.asarray(x, dtype=np.float32))
    kexp = np.asarray(basis, np.float32) @ np.asarray(kernel, np.float32)

    in_maps = []
    for h in range(NUM_HEADS):
        x_dev, w_dev = _host_prep(x, kexp, h)
        in_maps.append({"x16": x_dev, "w16": w_dev})

    if _BASS_CACHE is None:
        _BASS_CACHE = _build_bass()
    nc = _BASS_CACHE

    LAST_RESULT = run_bass_kernel_spmd(
        nc,
        in_maps,
        core_ids=list(range(NUM_HEADS)),
        trace=bool(int(os.environ.get("KERNEL_TRACE", "0"))),
    )

    out = np.empty((BATCH, SEQ, CHAN), np.float32)
    for h in range(NUM_HEADS):
        o_dev = LAST_RESULT.results[h]["o16"].astype(np.float32)  # (mc, m, tb*n)
        out_h = o_dev.reshape(SEQ, TOK)
        out[:, :, h::NUM_HEADS] = out_h.reshape(SEQ, BATCH, CH).transpose(1, 0, 2)
    return out


# revision 21
# speedup vs baseline: 1.0936x; 1.0936x over previous
"""Raw-bass (manual semaphore) equivariant-linear kernel, v7.

Math: per head h, out[b,:,h::8] = M_h^T @ x[b,:,h::8] with M_h the
512x512 3D-circulant generated from (basis@kernel)[:,h]; only 4 distinct
128x128 blocks (d = (kc-mc) mod 4). One head per NeuronCore.

Hard-won HW rules (each one hangs or wedges a NeuronCore if violated):
  1. Every engine the program touches must have a Block body (BassBlock
     does not branch unused engines to the end barrier).
  2. Never let two engines copy different column ranges of one PSUM bank
     concurrently; more generally one PSUM bank = one agent at a time
     (PE column-split accumulation groups in one bank hang too).
  3. SBUF->DRAM DMAs must write full partition rows of the destination
     region (column-sliced partial-row output DMAs hang).
  4. SWDGE and HWDGE DMAs may not share a semaphore; SWDGE sems must
     start at 0.

Performance model (from perfetto traces):
  - ~7.2us fixed framework preamble; first DMA flow ~9.0us.
  - Input streaming is capped ~210GB/s per core (SEngine port mux, all 8
    cores streaming).  w+x0 (640KB) is the matmul-start critical path ->
    lands ~12.3-12.6us when split across SP/ACT HWDGE + POOL SWDGE rings
    and the rings stay exclusive to it (gates hold x1/x2/x3 back).
  - PE HAM clock-gate: 1.2GHz until ~3.4us of *uninterrupted* PE
    activity; any >~0.3us idle gap resets/demotes it.  11 N=512 dummy
    matmuls bridge block-entry (~8.2us) to x0-arrival with no gap.
  - DMA cost is per packet (per partition row); output rows are 4KB.
  - A ring is FIFO: the final output DMA must not queue behind a big
    earlier one (o2 rides gpsimd's empty ring).

Layouts (per core):
  x16 (4 tb, 128, 2048) fp16 : row p = [kc0|kc1|kc2|kc3] tokens of block tb
  w16 (128, 512) fp16        : row p = [d0|d1|d2|d3]
  o16 (4 mc, 128, 2048) fp16 : row m = [tb0|tb1|tb2|tb3] (4KB rows)
"""

import os
from contextlib import ExitStack

import numpy as np

NUM_HEADS = 8
BATCH = 32
SEQ = 512
CHAN = 512
CH = CHAN // NUM_HEADS
P = 128
NKC = 4
NMC = 4
TOK = BATCH * CH
NTB = 4
N_WARM = 11
WARM_N = 512

LAST_RESULT = None
_BASS_CACHE = None


def _build_bass():
    import concourse.bass as bass
    import concourse.mybir as mybir

    fp16 = mybir.dt.float16
    fp32 = mybir.dt.float32

    nc = bass.Bass()

    x_d = nc.dram_tensor("x16", [NTB, P, NKC * 512], fp16, kind="ExternalInput")
    w_d = nc.dram_tensor("w16", [P, 4 * P], fp16, kind="ExternalInput")
    o_d = nc.dram_tensor("o16", [NMC, P, NTB * 512], fp16, kind="ExternalOutput")

    ctx = ExitStack()
    with ctx:
        XT = [
            ctx.enter_context(nc.sbuf_tensor(f"x_{tb}", [P, NKC * 512], fp16))
            for tb in range(NTB)
        ]
        WT = ctx.enter_context(nc.sbuf_tensor("w_all", [P, 4 * P], fp16))
        warm_w = ctx.enter_context(nc.sbuf_tensor("warm_w", [P, WARM_N], fp16))
        OT = [
            ctx.enter_context(nc.sbuf_tensor(f"ot_{mc}", [P, NTB * 512], fp16))
            for mc in range(NMC)
        ]
        PS = [
            ctx.enter_context(nc.psum_tensor(f"ps_{i}", [P, 512], fp32))
            for i in range(8)
        ]

        sem_mm = ctx.enter_context(nc.semaphore("mm"))
        sem_cp = ctx.enter_context(nc.semaphore("cp"))
        sem_cpa = ctx.enter_context(nc.semaphore("cpa"))
        sem_warm = ctx.enter_context(nc.semaphore("warm"))
        sem_wa = ctx.enter_context(nc.semaphore("in_wa"))
        sem_wb = ctx.enter_context(nc.semaphore("in_wb"))
        sem_x0a1 = ctx.enter_context(nc.semaphore("in_x0a1"))
        sem_x0a2 = ctx.enter_context(nc.semaphore("in_x0a2"))
        sem_x0b = ctx.enter_context(nc.semaphore("in_x0b"))
        sem_x0c = ctx.enter_context(nc.semaphore("in_x0c"))
        sem_x1a = ctx.enter_context(nc.semaphore("in_x1a"))
        sem_x1b = ctx.enter_context(nc.semaphore("in_x1b"))
        sem_x2a = ctx.enter_context(nc.semaphore("in_x2a"))
        sem_x2b = ctx.enter_context(nc.semaphore("in_x2b"))
        sem_x3 = ctx.enter_context(nc.semaphore("in_x3"))
        sem_od = ctx.enter_context(nc.semaphore("od"))
        sem_odg = ctx.enter_context(nc.semaphore("odg"))

        # matmul schedule: (tb, d, mc, start, stop). d-major (weight reuse)
        # except the last tb, which is mc-major so the final psum groups
        # retire (and stream out) early.
        mm_order = []
        for tb in (0, 1, 2):
            for d in range(4):
                for mc in range(NMC):
                    mm_order.append((tb, d, mc, d == 0, d == 3))
        for mc in range(NMC):
            for d in range(4):
                mm_order.append((3, d, mc, d == 0, d == 3))

        def ps_tile(tb, mc):
            return PS[(tb % 2) * 4 + mc]

        grp_done = {}
        ngrp = 0
        for tb, d, mc, start, stop in mm_order:
            if stop:
                ngrp += 1
                grp_done[(tb, mc)] = ngrp

        # psum->sbuf copies: tb 0..2 split DVE (mc even) / ACT (mc odd);
        # tb 3 all on DVE.
        dve_copies = []  # (tb, mc)
        act_copies = []
        for tb in range(3):
            for mc in range(NMC):
                (dve_copies if mc % 2 == 0 else act_copies).append((tb, mc))
        for mc in range(4):
            dve_copies.append((3, mc))

        cp_idx_d = {t: i + 1 for i, t in enumerate(dve_copies)}
        cp_idx_a = {t: i + 1 for i, t in enumerate(act_copies)}

        with nc.Block() as block:

            @block.sync
            def _(sync):
                sync.dma_start(WT[:64], w_d[:64]).then_inc(sem_wa, 16)
                sync.dma_start(XT[0][:32], x_d[0][:32]).then_inc(sem_x0a1, 16)
                sync.dma_start(XT[0][32:64], x_d[0][32:64]).then_inc(
                    sem_x0a2, 16
                )
                # hold x1/x2 back until the other rings' x0 shares are in
                sync.wait_ge(sem_x0b, 16)
                sync.wait_ge(sem_x0c, 16)
                sync.dma_start(XT[1][:64], x_d[1][:64]).then_inc(sem_x1a, 16)
                sync.dma_start(XT[2][:64], x_d[2][:64]).then_inc(sem_x2a, 16)
                # out DMAs: mc0, then final mc3 top half
                sync.wait_ge(sem_cp, cp_idx_d[(3, 0)])
                sync.dma_start(o_d[0], OT[0][:]).then_inc(sem_od, 16)
                sync.wait_ge(sem_cpa, cp_idx_a[(2, 3)])
                sync.wait_ge(sem_cp, cp_idx_d[(3, 3)])
                sync.dma_start(o_d[3, :64], OT[3][:64]).then_inc(sem_od, 16)

            @block.scalar
            def _(scalar):
                scalar.dma_start(WT[64:], w_d[64:]).then_inc(sem_wb, 16)
                scalar.dma_start(XT[0][64:96], x_d[0][64:96]).then_inc(
                    sem_x0b, 16
                )
                scalar.wait_ge(sem_x0a1, 16)
                scalar.wait_ge(sem_x0c, 16)
                scalar.dma_start(XT[1][64:], x_d[1][64:]).then_inc(sem_x1b, 16)
                scalar.dma_start(XT[2][64:], x_d[2][64:]).then_inc(sem_x2b, 16)
                # ACT copies for tb 0..2 (mc odd)
                for tb, mc in act_copies:
                    scalar.wait_ge(sem_mm, grp_done[(tb, mc)])
                    nc.scalar.copy(
                        OT[mc][:, tb * 512 : (tb + 1) * 512],
                        ps_tile(tb, mc)[:],
                    ).then_inc(sem_cpa, 1)
                # out DMA mc1, then final mc3 bottom half
                scalar.wait_ge(sem_cpa, cp_idx_a[(2, 1)])
                scalar.wait_ge(sem_cp, cp_idx_d[(3, 1)])
                scalar.dma_start(o_d[1], OT[1][:]).then_inc(sem_od, 16)
                scalar.wait_ge(sem_cpa, cp_idx_a[(2, 3)])
                scalar.wait_ge(sem_cp, cp_idx_d[(3, 3)])
                scalar.dma_start(o_d[3, 64:], OT[3][64:]).then_inc(sem_od, 16)

            @block.gpsimd
            def _(gpsimd):
                gpsimd.memset(warm_w[:], 0.0).then_inc(sem_warm, 1)
                gpsimd.dma_start(XT[0][96:], x_d[0][96:]).then_inc(sem_x0c, 16)
                gpsimd.wait_ge(sem_x0a1, 16)
                gpsimd.wait_ge(sem_x0b, 16)
                gpsimd.dma_start(XT[3][:], x_d[3]).then_inc(sem_x3, 16)
                # out DMA mc2 rides gpsimd's otherwise-empty ring so the
                # final mc3 DMAs don't queue behind it on sync/scalar
                gpsimd.wait_ge(sem_cp, cp_idx_d[(3, 2)])
                gpsimd.dma_start(o_d[2], OT[2][:]).then_inc(sem_odg, 16)

            @block.tensor
            def _(tensor):
                # HAM warm-up: dummy matmuls bridge block entry to x0
                # arrival with no PE idle gap (a gap resets the clock
                # gate).  Results go to PS[7]; every real group writing
                # PS[7] opens with start=True.
                tensor.wait_ge(sem_warm, 1)
                for _ in range(N_WARM):
                    nc.tensor.matmul(
                        PS[7][:],
                        warm_w[:, :P],
                        warm_w[:],
                        start=True,
                        stop=True,
                        skip_group_check=True,
                    )
                tensor.wait_ge(sem_wa, 16)
                tensor.wait_ge(sem_wb, 16)
                tensor.wait_ge(sem_x0a1, 16)
                tensor.wait_ge(sem_x0a2, 16)
                tensor.wait_ge(sem_x0b, 16)
                tensor.wait_ge(sem_x0c, 16)
                xsem = {1: (sem_x1a, sem_x1b), 2: (sem_x2a, sem_x2b),
                        3: (sem_x3,)}
                cur_tb = 0
                for tb, d, mc, start, stop in mm_order:
                    kc = (mc + d) % NKC
                    if tb != cur_tb:
                        for s in xsem[tb]:
                            tensor.wait_ge(s, 16)
                        if tb >= 2:
                            # WAR: psum banks reused from tb-2; wait for
                            # that tb's copies on both engines
                            tensor.wait_ge(
                                sem_cp,
                                max(
                                    cp_idx_d[(t, m)]
                                    for (t, m) in dve_copies
                                    if t == tb - 2
                                ),
                            )
                            tensor.wait_ge(
                                sem_cpa,
                                max(
                                    cp_idx_a[(t, m)]
                                    for (t, m) in act_copies
                                    if t == tb - 2
                                ),
                            )
                        cur_tb = tb
                    mm = nc.tensor.matmul(
                        ps_tile(tb, mc)[:],
                        WT[:, d * P : (d + 1) * P],
                        XT[tb][:, kc * 512 : (kc + 1) * 512],
                        start=start,
                        stop=stop,
                        skip_group_check=True,
                    )
                    if stop:
                        mm.then_inc(sem_mm, 1)

            @block.vector
            def _(vector):
                for tb, mc in dve_copies:
                    vector.wait_ge(sem_mm, grp_done[(tb, mc)])
                    nc.vector.tensor_copy(
                        OT[mc][:, tb * 512 : (tb + 1) * 512],
                        ps_tile(tb, mc)[:],
                    ).then_inc(sem_cp, 1)

    return nc


def _weight_tiles(kexp_h):
    w3 = kexp_h.reshape(8, 8, 8)
    p = np.arange(P)
    m = np.arange(P)
    dj = ((p[:, None] // 8) % 8 - (m[None, :] // 8) % 8) % 8
    dk = (p[:, None] % 8 - m[None, :] % 8) % 8
    tiles = np.empty((4, P, P), np.float32)
    for d in range(4):
        di = (2 * d + p[:, None] // 64 - m[None, :] // 64) % 8
        tiles[d] = w3[di, dj, dk]
    return tiles


def _host_prep(x, kexp, h):
    xh = x[:, :, h::NUM_HEADS]  # (32, 512, 64)
    x_dev = (
        xh.transpose(1, 0, 2)        # (g'', b, c)
        .reshape(NKC, P, NTB, 512)   # (kc, p, tb, n)
        .transpose(2, 1, 0, 3)       # (tb, p, kc, n)
        .reshape(NTB, P, NKC * 512)
        .astype(np.float16)
    )
    w_dev = (
        _weight_tiles(kexp[:, h])    # (d, p, m)
        .transpose(1, 0, 2)          # (p, d, m)
        .reshape(P, 4 * P)
        .astype(np.float16)
    )
    return np.ascontiguousarray(x_dev), np.ascontiguousarray(w_dev)


def kernel(x, basis, kernel):
    global LAST_RESULT, _BASS_CACHE
    from concourse.bass_utils import run_bass_kernel_spmd

    x = np.ascontiguousarray(np.asarray(x, dtype=np.float32))
    kexp = np.asarray(basis, np.float32) @ np.asarray(kernel, np.float32)

    in_maps = []
    for h in range(NUM_HEADS):
        x_dev, w_dev = _host_prep(x, kexp, h)
        in_maps.append({"x16": x_dev, "w16": w_dev})

    if _BASS_CACHE is None:
        _BASS_CACHE = _build_bass()
    nc = _BASS_CACHE

    LAST_RESULT = run_bass_kernel_spmd(
        nc,
        in_maps,
        core_ids=list(range(NUM_HEADS)),
        trace=bool(int(os.environ.get("KERNEL_TRACE", "0"))),
    )

    out = np.empty((BATCH, SEQ, CHAN), np.float32)
    for h in range(NUM_HEADS):
        o_dev = LAST_RESULT.results[h]["o16"].astype(np.float32)  # (mc, m, tb*n)
        out_h = o_dev.reshape(SEQ, TOK)
        out[:, :, h::NUM_HEADS] = out_h.reshape(SEQ, BATCH, CH).transpose(1, 0, 2)
    return out


# revision 22
# speedup vs baseline: 1.2140x; 1.1101x over previous
"""Raw-bass (manual semaphore) equivariant-linear kernel, v7.

Math: per head h, out[b,:,h::8] = M_h^T @ x[b,:,h::8] with M_h the
512x512 3D-circulant generated from (basis@kernel)[:,h]; only 4 distinct
128x128 blocks (d = (kc-mc) mod 4). One head per NeuronCore.

Hard-won HW rules (each one hangs or wedges a NeuronCore if violated):
  1. Every engine the program touches must have a Block body (BassBlock
     does not branch unused engines to the end barrier).
  2. Never let two engines copy different column ranges of one PSUM bank
     concurrently; more generally one PSUM bank = one agent at a time
     (PE column-split accumulation groups in one bank hang too).
  3. SBUF->DRAM DMAs must write full partition rows of the destination
     region (column-sliced partial-row output DMAs hang).
  4. SWDGE and HWDGE DMAs may not share a semaphore; SWDGE sems must
     start at 0.

Performance model (from perfetto traces):
  - ~7.2us fixed framework preamble; first DMA flow ~9.0us.
  - Input streaming is capped ~210GB/s per core (SEngine port mux, all 8
    cores streaming).  w+x0 (640KB) is the matmul-start critical path ->
    lands ~12.3-12.6us when split across SP/ACT HWDGE + POOL SWDGE rings
    and the rings stay exclusive to it (gates hold x1/x2/x3 back).
  - PE HAM clock-gate: 1.2GHz until ~3.4us of *uninterrupted* PE
    activity; any >~0.3us idle gap resets/demotes it.  11 N=512 dummy
    matmuls bridge block-entry (~8.2us) to x0-arrival with no gap.
  - DMA cost is per packet (per partition row); output rows are 4KB.
  - A ring is FIFO: the final output DMA must not queue behind a big
    earlier one (o2 rides gpsimd's empty ring).

Layouts (per core):
  x16 (4 tb, 128, 2048) fp16 : row p = [kc0|kc1|kc2|kc3] tokens of block tb
  w16 (128, 512) fp16        : row p = [d0|d1|d2|d3]
  o16 (4 mc, 128, 2048) fp16 : row m = [tb0|tb1|tb2|tb3] (4KB rows)
"""

import os
from contextlib import ExitStack

import numpy as np

NUM_HEADS = 8
BATCH = 32
SEQ = 512
CHAN = 512
CH = CHAN // NUM_HEADS
P = 128
NKC = 4
NMC = 4
TOK = BATCH * CH
NTB = 4
N_WARM = 13
WARM_N = 512

LAST_RESULT = None
_BASS_CACHE = None


def _build_bass():
    import concourse.bass as bass
    import concourse.mybir as mybir

    fp16 = mybir.dt.float16
    fp32 = mybir.dt.float32

    nc = bass.Bass()

    x_d = nc.dram_tensor("x16", [NTB, P, NKC * 512], fp16, kind="ExternalInput")
    w_d = nc.dram_tensor("w16", [P, 4 * P], fp16, kind="ExternalInput")
    o_d = nc.dram_tensor("o16", [NMC, P, NTB * 512], fp16, kind="ExternalOutput")

    ctx = ExitStack()
    with ctx:
        XT = [
            ctx.enter_context(nc.sbuf_tensor(f"x_{tb}", [P, NKC * 512], fp16))
            for tb in range(NTB)
        ]
        WT = ctx.enter_context(nc.sbuf_tensor("w_all", [P, 4 * P], fp16))
        warm_w = ctx.enter_context(nc.sbuf_tensor("warm_w", [P, WARM_N], fp16))
        OT = [
            ctx.enter_context(nc.sbuf_tensor(f"ot_{mc}", [P, NTB * 512], fp16))
            for mc in range(NMC)
        ]
        PS = [
            ctx.enter_context(nc.psum_tensor(f"ps_{i}", [P, 512], fp32))
            for i in range(8)
        ]

        sem_mm = ctx.enter_context(nc.semaphore("mm"))
        sem_cp = ctx.enter_context(nc.semaphore("cp"))
        sem_cpa = ctx.enter_context(nc.semaphore("cpa"))
        sem_warm = ctx.enter_context(nc.semaphore("warm"))
        sem_wa = ctx.enter_context(nc.semaphore("in_wa"))
        sem_wb = ctx.enter_context(nc.semaphore("in_wb"))
        sem_x0a1 = ctx.enter_context(nc.semaphore("in_x0a1"))
        sem_x0a2 = ctx.enter_context(nc.semaphore("in_x0a2"))
        sem_x0b = ctx.enter_context(nc.semaphore("in_x0b"))
        sem_x0c = ctx.enter_context(nc.semaphore("in_x0c"))
        sem_x1a = ctx.enter_context(nc.semaphore("in_x1a"))
        sem_x1b = ctx.enter_context(nc.semaphore("in_x1b"))
        sem_x2a = ctx.enter_context(nc.semaphore("in_x2a"))
        sem_x2b = ctx.enter_context(nc.semaphore("in_x2b"))
        sem_x3 = ctx.enter_context(nc.semaphore("in_x3"))
        sem_od = ctx.enter_context(nc.semaphore("od"))
        sem_x2c = ctx.enter_context(nc.semaphore("in_x2c"))

        # matmul schedule: (tb, d, mc, start, stop). d-major (weight reuse)
        # except the last tb, which is mc-major so the final psum groups
        # retire (and stream out) early.
        # entries: (tb, d, mc, start, stop, half) -- half is None for a
        # full-width group, else ('a'|'b') column half of (3,3).  Half 'b'
        # lands in bank 4 (freed by (3,0)'s copy) so its matmuls can run
        # while DVE copies half 'a' out of bank 7 (one agent per bank).
        mm_order = []
        for tb in (0, 1, 2):
            for d in range(4):
                for mc in range(NMC):
                    mm_order.append((tb, d, mc, d == 0, d == 3, None))
        for mc in range(3):
            for d in range(4):
                mm_order.append((3, d, mc, d == 0, d == 3, None))
        for half in ("a", "b"):
            for d in range(4):
                mm_order.append((3, d, 3, d == 0, d == 3, half))

        def ps_ap(tb, mc, half):
            if half is None:
                return PS[(tb % 2) * 4 + mc][:]
            if half == "a":
                return PS[7][:, 0:256]
            return PS[4][:, 0:256]

        grp_done = {}
        ngrp = 0
        for tb, d, mc, start, stop, half in mm_order:
            if stop:
                ngrp += 1
                grp_done[(tb, mc, half)] = ngrp

        # psum->sbuf copies: tb 0..2 split DVE (mc even) / ACT (mc odd);
        # tb 3 all on DVE, the (3,3) halves as separate copies.
        dve_copies = []  # (tb, mc, half)
        act_copies = []
        for tb in range(3):
            for mc in range(NMC):
                (dve_copies if mc % 2 == 0 else act_copies).append(
                    (tb, mc, None)
                )
        for mc in range(3):
            dve_copies.append((3, mc, None))
        dve_copies.append((3, 3, "a"))
        dve_copies.append((3, 3, "b"))

        cp_idx_d = {t: i + 1 for i, t in enumerate(dve_copies)}
        cp_idx_a = {t: i + 1 for i, t in enumerate(act_copies)}

        with nc.Block() as block:

            @block.sync
            def _(sync):
                sync.dma_start(WT[:64], w_d[:64]).then_inc(sem_wa, 16)
                sync.dma_start(XT[0][:32], x_d[0][:32]).then_inc(sem_x0a1, 16)
                sync.dma_start(XT[0][32:64], x_d[0][32:64]).then_inc(
                    sem_x0a2, 16
                )
                # hold x1/x2 back until the other rings' x0 shares are in
                sync.wait_ge(sem_x0b, 16)
                sync.wait_ge(sem_x0c, 16)
                sync.dma_start(XT[1][:64], x_d[1][:64]).then_inc(sem_x1a, 16)
                sync.dma_start(XT[2][:48], x_d[2][:48]).then_inc(sem_x2a, 16)
                # out DMAs: mc0, then final mc3 top half
                sync.wait_ge(sem_cp, cp_idx_d[(3, 0, None)])
                sync.dma_start(o_d[0], OT[0][:]).then_inc(sem_od, 16)
                sync.wait_ge(sem_cpa, cp_idx_a[(2, 3, None)])
                sync.wait_ge(sem_cp, cp_idx_d[(3, 3, "b")])
                sync.dma_start(o_d[3, :64], OT[3][:64]).then_inc(sem_od, 16)

            @block.scalar
            def _(scalar):
                scalar.dma_start(WT[64:], w_d[64:]).then_inc(sem_wb, 16)
                scalar.dma_start(XT[0][64:96], x_d[0][64:96]).then_inc(
                    sem_x0b, 16
                )
                scalar.wait_ge(sem_x0a1, 16)
                scalar.wait_ge(sem_x0c, 16)
                scalar.dma_start(XT[1][64:], x_d[1][64:]).then_inc(sem_x1b, 16)
                scalar.dma_start(XT[2][48:96], x_d[2][48:96]).then_inc(sem_x2b, 16)
                # ACT copies for tb 0..2 (mc odd)
                for tb, mc, half in act_copies:
                    scalar.wait_ge(sem_mm, grp_done[(tb, mc, half)])
                    nc.scalar.copy(
                        OT[mc][:, tb * 512 : (tb + 1) * 512],
                        ps_ap(tb, mc, half),
                    ).then_inc(sem_cpa, 1)
                # out DMA mc1, then final mc3 bottom half
                scalar.wait_ge(sem_cpa, cp_idx_a[(2, 1, None)])
                scalar.wait_ge(sem_cp, cp_idx_d[(3, 1, None)])
                scalar.dma_start(o_d[1], OT[1][:]).then_inc(sem_od, 16)
                scalar.wait_ge(sem_cp, cp_idx_d[(3, 2, None)])
                scalar.dma_start(o_d[2], OT[2][:]).then_inc(sem_od, 16)
                scalar.wait_ge(sem_cpa, cp_idx_a[(2, 3, None)])
                scalar.wait_ge(sem_cp, cp_idx_d[(3, 3, "b")])
                scalar.dma_start(o_d[3, 64:], OT[3][64:]).then_inc(sem_od, 16)

            @block.gpsimd
            def _(gpsimd):
                gpsimd.memset(warm_w[:], 0.0).then_inc(sem_warm, 1)
                gpsimd.dma_start(XT[0][96:], x_d[0][96:]).then_inc(sem_x0c, 16)
                gpsimd.wait_ge(sem_x0a1, 16)
                gpsimd.wait_ge(sem_x0b, 16)
                gpsimd.dma_start(XT[2][96:], x_d[2][96:]).then_inc(sem_x2c, 16)
                gpsimd.dma_start(XT[3][:], x_d[3]).then_inc(sem_x3, 16)

            @block.tensor
            def _(tensor):
                # HAM warm-up: dummy matmuls bridge block entry to x0
                # arrival with no PE idle gap (a gap resets the clock
                # gate).  Results go to PS[7]; every real group writing
                # PS[7] opens with start=True.
                tensor.wait_ge(sem_warm, 1)
                for _ in range(N_WARM):
                    nc.tensor.matmul(
                        PS[7][:],
                        warm_w[:, :P],
                        warm_w[:],
                        start=True,
                        stop=True,
                        skip_group_check=True,
                    )
                tensor.wait_ge(sem_wa, 16)
                tensor.wait_ge(sem_wb, 16)
                tensor.wait_ge(sem_x0a1, 16)
                tensor.wait_ge(sem_x0a2, 16)
                tensor.wait_ge(sem_x0b, 16)
                tensor.wait_ge(sem_x0c, 16)
                xsem = {1: (sem_x1a, sem_x1b),
                        2: (sem_x2a, sem_x2b, sem_x2c), 3: (sem_x3,)}
                cur_tb = 0
                for tb, d, mc, start, stop, half in mm_order:
                    kc = (mc + d) % NKC
                    if tb != cur_tb:
                        for s in xsem[tb]:
                            tensor.wait_ge(s, 16)
                        if tb >= 2:
                            # WAR: psum banks reused from tb-2; wait for
                            # that tb's copies on both engines
                            tensor.wait_ge(
                                sem_cp,
                                max(
                                    cp_idx_d[(t, m, h)]
                                    for (t, m, h) in dve_copies
                                    if t == tb - 2
                                ),
                            )
                            tensor.wait_ge(
                                sem_cpa,
                                max(
                                    cp_idx_a[(t, m, h)]
                                    for (t, m, h) in act_copies
                                    if t == tb - 2
                                ),
                            )
                        cur_tb = tb
                    if half == "b" and start:
                        # WAR: half b reuses bank 4 after (3,0)'s copy
                        tensor.wait_ge(sem_cp, cp_idx_d[(3, 0, None)])
                    lo = 256 if half == "b" else 0
                    hi = 256 if half == "a" else 512
                    mm = nc.tensor.matmul(
                        ps_ap(tb, mc, half),
                        WT[:, d * P : (d + 1) * P],
                        XT[tb][:, kc * 512 + lo : kc * 512 + hi],
                        start=start,
                        stop=stop,
                        skip_group_check=True,
                    )
                    if stop:
                        mm.then_inc(sem_mm, 1)

            @block.vector
            def _(vector):
                for tb, mc, half in dve_copies:
                    vector.wait_ge(sem_mm, grp_done[(tb, mc, half)])
                    lo = 256 if half == "b" else 0
                    hi = 256 if half == "a" else 512
                    nc.vector.tensor_copy(
                        OT[mc][:, tb * 512 + lo : tb * 512 + hi],
                        ps_ap(tb, mc, half),
                    ).then_inc(sem_cp, 1)

    return nc


def _weight_tiles(kexp_h):
    w3 = kexp_h.reshape(8, 8, 8)
    p = np.arange(P)
    m = np.arange(P)
    dj = ((p[:, None] // 8) % 8 - (m[None, :] // 8) % 8) % 8
    dk = (p[:, None] % 8 - m[None, :] % 8) % 8
    tiles = np.empty((4, P, P), np.float32)
    for d in range(4):
        di = (2 * d + p[:, None] // 64 - m[None, :] // 64) % 8
        tiles[d] = w3[di, dj, dk]
    return tiles


def _host_prep(x, kexp, h):
    xh = x[:, :, h::NUM_HEADS]  # (32, 512, 64)
    x_dev = (
        xh.transpose(1, 0, 2)        # (g'', b, c)
        .reshape(NKC, P, NTB, 512)   # (kc, p, tb, n)
        .transpose(2, 1, 0, 3)       # (tb, p, kc, n)
        .reshape(NTB, P, NKC * 512)
        .astype(np.float16)
    )
    w_dev = (
        _weight_tiles(kexp[:, h])    # (d, p, m)
        .transpose(1, 0, 2)          # (p, d, m)
        .reshape(P, 4 * P)
        .astype(np.float16)
    )
    return np.ascontiguousarray(x_dev), np.ascontiguousarray(w_dev)


def kernel(x, basis, kernel):
    global LAST_RESULT, _BASS_CACHE
    from concourse.bass_utils import run_bass_kernel_spmd

    x = np.ascontiguousarray(np.asarray(x, dtype=np.float32))
    kexp = np.asarray(basis, np.float32) @ np.asarray(kernel, np.float32)

    in_maps = []
    for h in range(NUM_HEADS):
        x_dev, w_dev = _host_prep(x, kexp, h)
        in_maps.append({"x16": x_dev, "w16": w_dev})

    if _BASS_CACHE is None:
        _BASS_CACHE = _build_bass()
    nc = _BASS_CACHE

    LAST_RESULT = run_bass_kernel_spmd(
        nc,
        in_maps,
        core_ids=list(range(NUM_HEADS)),
        trace=bool(int(os.environ.get("KERNEL_TRACE", "0"))),
    )

    out = np.empty((BATCH, SEQ, CHAN), np.float32)
    for h in range(NUM_HEADS):
        o_dev = LAST_RESULT.results[h]["o16"].astype(np.float32)  # (mc, m, tb*n)
        out_h = o_dev.reshape(SEQ, TOK)
        out[:, :, h::NUM_HEADS] = out_h.reshape(SEQ, BATCH, CH).transpose(1, 0, 2)
    return out


# revision 23
# speedup vs baseline: 1.2829x; 1.0568x over previous
"""Raw-bass (manual semaphore) variant of the equivariant-linear kernel.

Math: per head h, out[b,:,h::8] = M_h^T @ x[b,:,h::8] with M_h the
512x512 3D-circulant generated from (basis@kernel)[:,h]; only 4 distinct
128x128 blocks (d = (kc-mc) mod 4). One head per NeuronCore.

Layouts (per core):
  x16 (4 tb, 128, 2048) fp16 : row p = [kc0|kc1|kc2|kc3] tokens of block tb
  w16 (128, 512) fp16        : row p = [d0|d1|d2|d3]
  o16 (4 mc, 4 tb, 128, 512) fp16
4KB-per-partition-row input DMAs (DMA cost is per packet, not per byte).
"""

import os
from contextlib import ExitStack

import numpy as np

NUM_HEADS = 8
BATCH = 32
SEQ = 512
CHAN = 512
CH = CHAN // NUM_HEADS
P = 128
NKC = 4
NMC = 4
TOK = BATCH * CH
NTB = 4
N_WARM = 10

LAST_RESULT = None
_BASS_CACHE = None


def _build_bass():
    import concourse.bass as bass
    import concourse.mybir as mybir

    fp16 = mybir.dt.float16
    fp32 = mybir.dt.float32

    nc = bass.Bass()

    x_d = nc.dram_tensor("x16", [NTB, P, NKC * 512], fp16, kind="ExternalInput")
    w_d = nc.dram_tensor("w16", [P, 4 * P], fp16, kind="ExternalInput")
    o_d = nc.dram_tensor("o16", [NMC, NTB, P, 512], fp16, kind="ExternalOutput")

    ctx = ExitStack()
    with ctx:
        XT = [
            ctx.enter_context(nc.sbuf_tensor(f"x_{tb}", [P, NKC * 512], fp16))
            for tb in range(NTB)
        ]
        warm_w = ctx.enter_context(nc.sbuf_tensor("warm_w", [P, 512], fp16))
        WT = ctx.enter_context(nc.sbuf_tensor("w_all", [P, 4 * P], fp16))
        OT = [
            ctx.enter_context(nc.sbuf_tensor(f"ot_{i}", [P, 512], fp16))
            for i in range(16)
        ]
        PS = [
            ctx.enter_context(nc.psum_tensor(f"ps_{i}", [P, 512], fp32))
            for i in range(8)
        ]

        sem_mm = ctx.enter_context(nc.semaphore("mm"))
        sem_cp = ctx.enter_context(nc.semaphore("cp"))
        sem_cpa = ctx.enter_context(nc.semaphore("cpa"))
        sem_od = ctx.enter_context(nc.semaphore("od"))
        sem_od_sw = ctx.enter_context(nc.semaphore("od_sw"))
        sem_warm = ctx.enter_context(nc.semaphore("warm"))
        sem_wa = ctx.enter_context(nc.semaphore("in_wa"))
        sem_wb = ctx.enter_context(nc.semaphore("in_wb"))
        sem_x0t = ctx.enter_context(nc.semaphore("in_x0t"))
        sem_x0b = ctx.enter_context(nc.semaphore("in_x0b"))
        sem_x1 = ctx.enter_context(nc.semaphore("in_x1"))
        sem_x2 = ctx.enter_context(nc.semaphore("in_x2"))
        sem_x3 = ctx.enter_context(nc.semaphore("in_x3"))

        # matmul schedule: (tb, d, mc, start, stop). d-major (weight reuse)
        # except the last tb, which is mc-major so the final psum groups
        # retire (and copy out) early.
        mm_order = []
        for tb in (0, 1, 2):
            for d in range(4):
                for mc in range(NMC):
                    mm_order.append((tb, d, mc, d == 0, d == 3))
        for mc in range(NMC):
            for d in range(4):
                mm_order.append((3, d, mc, d == 0, d == 3))

        def ps_tile(tb, mc):
            return PS[(tb % 2) * 4 + mc]

        grp_done = {}
        ngrp = 0
        for tb, d, mc, start, stop in mm_order:
            if stop:
                ngrp += 1
                grp_done[(tb, mc)] = ngrp

        # out-DMA queue assignment: early blocks round-robin all 3 queues,
        # final block only on the low-latency HWDGE queues.
        def out_queue(i):
            if i >= 12:
                return ("sync", "scalar")[i % 2]
            return ("gpsimd", "sync", "scalar")[i % 3]

        # psum->sbuf copies split across DVE and ACT, each with its own
        # ordered count semaphore. Last tb: ACT (2x slower) takes the early
        # groups, DVE the final two, so the tail is short.
        def copy_engine(tb, mc):
            if tb == NTB - 1:
                return "dve"
            return "dve" if mc % 2 == 0 else "act"

        cp_count = {}
        ndve = nact = 0
        for tb in range(NTB):
            for mc in range(NMC):
                if copy_engine(tb, mc) == "dve":
                    ndve += 1
                    cp_count[(tb, mc)] = ("dve", ndve)
                else:
                    nact += 1
                    cp_count[(tb, mc)] = ("act", nact)

        def wait_copy(eng, tb, mc):
            which, cnt = cp_count[(tb, mc)]
            eng.wait_ge(sem_cp if which == "dve" else sem_cpa, cnt)

        with nc.Block() as block:

            @block.sync
            def _(sync):
                sync.dma_start(WT[:64], w_d[:64]).then_inc(sem_wa, 16)
                sync.dma_start(XT[0][:64], x_d[0][:64]).then_inc(sem_x0t, 16)
                sync.dma_start(XT[2][:], x_d[2]).then_inc(sem_x2, 16)
                for tb in range(NTB):
                    for mc in range(NMC):
                        i = tb * 4 + mc
                        if out_queue(i) == "sync":
                            wait_copy(sync, tb, mc)
                            sync.dma_start(o_d[mc, tb], OT[i][:]).then_inc(sem_od, 16)

            @block.scalar
            def _(scalar):
                scalar.dma_start(WT[64:], w_d[64:]).then_inc(sem_wb, 16)
                scalar.dma_start(XT[0][64:], x_d[0][64:]).then_inc(sem_x0b, 16)
                scalar.dma_start(XT[1][:], x_d[1]).then_inc(sem_x1, 16)
                scalar.wait_ge(sem_x1, 16)
                scalar.dma_start(XT[3][:], x_d[3]).then_inc(sem_x3, 16)
                for tb in range(NTB):
                    for mc in range(NMC):
                        i = tb * 4 + mc
                        if copy_engine(tb, mc) == "act":
                            scalar.wait_ge(sem_mm, grp_done[(tb, mc)])
                            nc.scalar.copy(OT[i][:], ps_tile(tb, mc)[:]).then_inc(
                                sem_cpa, 1
                            )
                        if out_queue(i) == "scalar":
                            wait_copy(scalar, tb, mc)
                            scalar.dma_start(o_d[mc, tb], OT[i][:]).then_inc(
                                sem_od, 16
                            )

            @block.gpsimd
            def _(gpsimd):
                gpsimd.memset(warm_w[:], 0.0).then_inc(sem_warm, 1)
                for tb in range(NTB):
                    for mc in range(NMC):
                        i = tb * 4 + mc
                        if out_queue(i) == "gpsimd":
                            wait_copy(gpsimd, tb, mc)
                            gpsimd.dma_start(o_d[mc, tb], OT[i][:]).then_inc(
                                sem_od_sw, 16
                            )

            @block.tensor
            def _(tensor):
                # HAM warm-up: full-width matmuls (N=512) on a zeroed tile
                # so the clock-gate sees real PE activity while inputs stream.
                tensor.wait_ge(sem_warm, 1)
                for _ in range(N_WARM):
                    nc.tensor.matmul(
                        PS[7][:], warm_w[:, :P], warm_w[:], start=True, stop=True,
                        skip_group_check=True,
                    )
                tensor.wait_ge(sem_wa, 16)
                tensor.wait_ge(sem_wb, 16)
                tensor.wait_ge(sem_x0t, 16)
                tensor.wait_ge(sem_x0b, 16)
                xsem = {1: sem_x1, 2: sem_x2, 3: sem_x3}
                cur_tb = 0
                for tb, d, mc, start, stop in mm_order:
                    kc = (mc + d) % NKC
                    if tb != cur_tb:
                        tensor.wait_ge(xsem[tb], 16)
                        if tb >= 2:
                            # WAR: psum banks reused from tb-2; count copies
                            # of tb-2 done per engine
                            ndv = sum(1 for t in range(tb - 1) for m in range(NMC)
                                      if copy_engine(t, m) == "dve")
                            nac = sum(1 for t in range(tb - 1) for m in range(NMC)
                                      if copy_engine(t, m) == "act")
                            tensor.wait_ge(sem_cp, ndv)
                            tensor.wait_ge(sem_cpa, nac)
                        cur_tb = tb
                    mm = nc.tensor.matmul(
                        ps_tile(tb, mc)[:],
                        WT[:, d * P:(d + 1) * P],
                        XT[tb][:, kc * 512:(kc + 1) * 512],
                        start=start,
                        stop=stop,
                        skip_group_check=True,
                    )
                    if stop:
                        mm.then_inc(sem_mm, 1)

            @block.vector
            def _(vector):
                for tb in range(NTB):
                    for mc in range(NMC):
                        i = tb * 4 + mc
                        if copy_engine(tb, mc) == "dve":
                            vector.wait_ge(sem_mm, grp_done[(tb, mc)])
                            nc.vector.tensor_copy(
                                OT[i][:], ps_tile(tb, mc)[:]
                            ).then_inc(sem_cp, 1)

    return nc


def _weight_tiles(kexp_h):
    w3 = kexp_h.reshape(8, 8, 8)
    p = np.arange(P)
    m = np.arange(P)
    dj = ((p[:, None] // 8) % 8 - (m[None, :] // 8) % 8) % 8
    dk = (p[:, None] % 8 - m[None, :] % 8) % 8
    tiles = np.empty((4, P, P), np.float32)
    for d in range(4):
        di = (2 * d + p[:, None] // 64 - m[None, :] // 64) % 8
        tiles[d] = w3[di, dj, dk]
    return tiles


def _host_prep(x, kexp, h):
    xh = x[:, :, h::NUM_HEADS]  # (32, 512, 64)
    x_dev = (
        xh.transpose(1, 0, 2)        # (g'', b, c)
        .reshape(NKC, P, NTB, 512)   # (kc, p, tb, n)
        .transpose(2, 1, 0, 3)       # (tb, p, kc, n)
        .reshape(NTB, P, NKC * 512)
        .astype(np.float16)
    )
    w_dev = (
        _weight_tiles(kexp[:, h])    # (d, p, m)
        .transpose(1, 0, 2)          # (p, d, m)
        .reshape(P, 4 * P)
        .astype(np.float16)
    )
    return np.ascontiguousarray(x_dev), np.ascontiguousarray(w_dev)


def kernel(x, basis, kernel):
    global LAST_RESULT, _BASS_CACHE
    from concourse.bass_utils import run_bass_kernel_spmd

    x = np.ascontiguousarray(np.asarray(x, dtype=np.float32))
    kexp = np.asarray(basis, np.float32) @ np.asarray(kernel, np.float32)

    in_maps = []
    for h in range(NUM_HEADS):
        x_dev, w_dev = _host_prep(x, kexp, h)
        in_maps.append({"x16": x_dev, "w16": w_dev})

    if _BASS_CACHE is None:
        _BASS_CACHE = _build_bass()
    nc = _BASS_CACHE

    LAST_RESULT = run_bass_kernel_spmd(
        nc,
        in_maps,
        core_ids=list(range(NUM_HEADS)),
        trace=bool(int(os.environ.get("KERNEL_TRACE", "0"))),
    )

    out = np.empty((BATCH, SEQ, CHAN), np.float32)
    for h in range(NUM_HEADS):
        o_dev = LAST_RESULT.results[h]["o16"].astype(np.float32)  # (mc, tb, m, n)
        out_h = o_dev.transpose(0, 2, 1, 3).reshape(SEQ, TOK)
        out[:, :, h::NUM_HEADS] = out_h.reshape(SEQ, BATCH, CH).transpose(1, 0, 2)
    return out

